# revision 1
# baseline (speedup 1.0000x reference)
"""MinLSTM Trainium2 kernel.

Full-input contract: kernel(**inputs) takes the complete (unsharded) numpy
inputs of the reference model and returns the full [B, T+1, H] float32 output.

Math (per batch b, channel h — identical to the reference's log-space scan,
computed in linear space; every quantity is positive so the linear recurrence
is numerically stable):
    a = x @ W_f + b_f ;  b = x @ W_i + b_i ;  c = x @ W_h + b_h
    f = sigmoid(softplus(-b) - softplus(-a))        # forget gate
    i = 1 - f                                       # input gate
    g = max(c + 0.5, sigmoid(c))                    # = exp(log_g(c))
    h_t = f_t * h_{t-1} + i_t * g_t,   h_{-1} = g(h_0)
    out[:, 0] = g(h_0); out[:, t+1] = h_t

Sharding: 8 cores, core c -> (sample b = c//2, H-half hh = c%2, 256 channels).
Fully independent cores, no collectives. Host pre-transposes x to xT so the
device contraction dim (D) lies on partitions; host assembles the output.

Device pipeline per 512-wide T-chunk: DMA xT tiles -> matmuls (W stationary,
xT moving, PSUM fp32, [h, t] layout) -> ScalarE softplus/sigmoid gates ->
VectorE elementwise -> tensor_tensor_scan (fp32 state) -> DMA out.
"""

from contextlib import ExitStack

import numpy as np
import ml_dtypes

import concourse.bacc as bacc
import concourse.tile as tile
import concourse.mybir as mybir
from concourse.bass_utils import run_bass_kernel_spmd

# ---- fused custom DVE op: r = ~1/(in0+in1) --------------------------------
# One 8-slice pass: x = in0+in1; nx = bitcast(~x) (exponent-flip seed);
# u = x*nx lands in [-4.5,-4]; r = nx * p2(u) with p2 a degree-2 minimax of
# 1/u on that interval. Max rel err ~5.2e-5. Replaces a GpSimd add +
# reciprocal_approx_fast pair.
import concourse.dve_ops as _dve_ops
from concourse.dve_spec import (Spec as _Spec, Src0 as _S0, Src1 as _S1,
                                C0 as _C0, C1 as _C1, C2 as _C2,
                                AluOp as _AluOp, Bin as _Bin, lower as _lower)
from concourse.dve_uop import DveOpSpec as _DveOpSpec
from concourse.dve_table_gen import dve_ver_for as _dve_ver_for

ADD_RECIP_CONSTS = {"s0": -0.01306049, "s1": -0.16652115, "imm2": -0.70710396}


def _register_add_recip():
    name = "ADD_RECIP_POLY2_ANT"
    if name in _dve_ops._SUB_OPCODE_FOR_NAME:
        return next(o for o in _dve_ops.OPS if o.name == name)
    _x = _S0 + _S1
    _nx = _Bin(_AluOp.BITWISE_NOT, _x, _x)
    _u = _x * _nx

    def _ref(in0, in1, c0, c1, c2):
        x = (np.asarray(in0, np.float32) + np.asarray(in1, np.float32))
        x = x.astype(np.float32)
        nx = (~x.view(np.int32)).view(np.float32)
        u = x * nx
        return ((u * c0 + c1) * u + c2) * nx

    spec = _Spec(body=((_u * _C0 + _C1) * _u + _C2) * _nx, reference=_ref)
    row = _dve_ops._CUSTOM_DVE_ROW_BASE + len(_dve_ops.OPS)
    assert row < 0x20
    ver = _dve_ver_for("TRN2")
    sha = _DveOpSpec(name=name, opcode=row, uops=_lower(spec, ver=ver),
                     rd1_en=True).sha(ver)
    op = _dve_ops.DveOp(name, spec, subdim=False, uops_sha={ver: sha})
    _dve_ops.OPS.append(op)
    _dve_ops.CUSTOM_DVE_SPECS[name] = spec
    _dve_ops._SUB_OPCODE_FOR_NAME[name] = row
    return op


_ADD_RECIP_OP = _register_add_recip()

BF = mybir.dt.bfloat16
F16 = mybir.dt.float16
F32 = mybir.dt.float32
F32R = mybir.dt.float32r
AF = mybir.ActivationFunctionType
OP = mybir.AluOpType

B, T, D, H = 4, 8192, 512, 512
NCORES = 8
HS = H // 2          # 256 channels per core
TC = 512             # T chunk width
NCH = T // TC        # 16 chunks
NDT = D // 128       # 4 contraction tiles
NHT = HS // 128      # 2 h-tiles per core

# Matmul input mode: "f32r" (fp32 data, full-rate replicated mode) or "bf16".
MM_MODE = "f32r"
# Gate tensor dtype on-chip (f16 halves DVE cost vs f32, ~8x less rounding
# than bf16; values are in [0, ~8] so fp16 range is ample).
GT = F16

_nc_cache = {}


def _build_nc(mm_mode=MM_MODE):
    mm_dt = F32R if mm_mode == "f32r" else BF
    nc = bacc.Bacc("TRN2", target_bir_lowering=False, debug=False,
                   num_devices=NCORES)
    xT = nc.dram_tensor("xT", [D, T], mm_dt, kind="ExternalInput")
    w = nc.dram_tensor("w", [D, 3 * HS], mm_dt, kind="ExternalInput")
    aux = nc.dram_tensor("aux", [128, NHT], F32, kind="ExternalInput")
    out = nc.dram_tensor("out", [HS, T], F32, kind="ExternalOutput")

    def mm_ap(t):
        return t

    with tile.TileContext(nc) as tc, ExitStack() as ctx:
        wpool = ctx.enter_context(tc.tile_pool(name="w", bufs=1))
        xpool = ctx.enter_context(tc.tile_pool(name="x", bufs=4))
        gpool = ctx.enter_context(tc.tile_pool(name="g", bufs=4))
        hpool = ctx.enter_context(tc.tile_pool(name="h", bufs=4))
        ppool = ctx.enter_context(tc.tile_pool(name="p", bufs=2, space="PSUM"))

        # weight/aux loads go out on the ACT HWDGE queue so the first x-chunk
        # loads (SP queue) run in parallel with them; one 3D-AP DMA covers all
        # four 128-row d-slices
        wts = []
        for dt_ in range(NDT):
            t_ = wpool.tile([128, 3 * HS], mm_dt, tag=f"w{dt_}", name=f"w{dt_}")
            nc.scalar.dma_start(t_[:], w[dt_ * 128:(dt_ + 1) * 128, :])
            wts.append(t_)
        auxt = wpool.tile([128, NHT], F32, tag="aux")
        nc.scalar.dma_start(auxt[:], aux[:])

        # chunk 0 split in half so the first matmul group starts on a
        # half-size x transfer
        chunks = [(0, TC // 2), (TC // 2, TC // 2)]
        chunks += [(k * TC, TC) for k in range(1, NCH)]

        carry = [None] * NHT
        for ci, (t0, tw) in enumerate(chunks):
            tsl = slice(t0, t0 + tw)
            xts = []
            for dt_ in range(NDT):
                xt = xpool.tile([128, TC], mm_dt, tag=f"x{dt_}", name=f"x{dt_}")
                nc.sync.dma_start(xt[:, :tw], xT[dt_ * 128:(dt_ + 1) * 128, tsl])
                xts.append(xt[:, :tw])
            for ht in range(NHT):
                # f_pre and i_pre share one two-bank PSUM tile so a single
                # ScalarE sigmoid covers both
                pfi = ppool.tile([128, 2, TC], F32, tag="pre01", bufs=2)
                pc_t = ppool.tile([128, TC], F32, tag="pre2", bufs=4)
                for wi in range(3):
                    dst = pc_t[:, :tw] if wi == 2 else pfi[:, wi, :tw]
                    for dt_ in range(NDT):
                        c0 = wi * HS + ht * 128
                        nc.tensor.matmul(
                            dst, mm_ap(wts[dt_][:, c0:c0 + 128]),
                            mm_ap(xts[dt_]),
                            start=(dt_ == 0), stop=(dt_ == NDT - 1))
                pc = pc_t[:, :tw]  # h_pre

                # f = sa/(sa+sb), i = sb/(sa+sb)  (exactly the reference's
                # sigmoid(softplus-difference) gates); g = max(c+.5, sigmoid(c))
                sab = gpool.tile([128, 2, TC], F32, tag="sab", name="sab")
                nc.scalar.activation(sab[:, :, :tw], pfi[:, :, :tw], AF.Sigmoid)
                sa = sab[:, 0, :tw]
                sb = sab[:, 1, :tw]
                sg = gpool.tile([128, TC], GT, tag="sg", name="sg")[:, :tw]
                nc.scalar.activation(sg, pc, AF.Sigmoid)
                # g emitted first on DVE so the h_pre PSUM bank frees early
                g = gpool.tile([128, TC], GT, tag="g", name="g")[:, :tw]
                nc.vector.scalar_tensor_tensor(g, pc, 0.5, sg, OP.add, OP.max)
                r = gpool.tile([128, TC], F32, tag="r", name="r")[:, :tw]
                c = ADD_RECIP_CONSTS
                nc.vector._custom_dve(_ADD_RECIP_OP, out=r, in0=sa,
                                      in1=sb, s0=c["s0"], s1=c["s1"],
                                      imm2=c["imm2"])
                f = gpool.tile([128, TC], GT, tag="f", name="f")[:, :tw]
                nc.gpsimd.tensor_tensor(f, sa, r, op=OP.mult)
                # f + (1-f) = (sa+sb)/s: the input gate is exactly 1-f
                w = gpool.tile([128, TC], GT, tag="w", name="w")[:, :tw]
                nc.vector.tensor_scalar(w, f, -1.0, 1.0, OP.mult, OP.add)
                v = gpool.tile([128, TC], GT, tag="v", name="v")[:, :tw]
                nc.vector.tensor_tensor(v, w, g, op=OP.mult)
                h = hpool.tile([128, TC], F32, tag=f"h{ht}",
                               name=f"h{ht}")[:, :tw]
                ini = auxt[:, ht:ht + 1] if ci == 0 else carry[ht]
                nc.vector.tensor_tensor_scan(h, f, v, ini, OP.mult, OP.add)
                carry[ht] = h[:, tw - 1:tw]
                nc.sync.dma_start(out[ht * 128:(ht + 1) * 128, tsl], h)
    nc.compile()
    return nc


def _get_nc(mm_mode=MM_MODE):
    if mm_mode not in _nc_cache:
        _nc_cache[mm_mode] = _build_nc(mm_mode)
    return _nc_cache[mm_mode]


def _g_host(x):
    # exp(log_g(x)) of the reference, computed directly in fp32
    return np.where(x >= 0, x + 0.5, 1.0 / (1.0 + np.exp(-np.minimum(x, 0))))


def _run(inputs, mm_mode=MM_MODE, trace=False):
    x = np.asarray(inputs["x"], np.float32)
    h_0 = np.asarray(inputs["h_0"], np.float32)
    W_f = np.asarray(inputs["W_f"], np.float32)
    b_f = np.asarray(inputs["b_f"], np.float32)
    W_i = np.asarray(inputs["W_i"], np.float32)
    b_i = np.asarray(inputs["b_i"], np.float32)
    W_h = np.asarray(inputs["W_h"], np.float32)
    b_h = np.asarray(inputs["b_h"], np.float32)
    assert (b_f == 0).all() and (b_i == 0).all() and (b_h == 0).all(), \
        "device program folds zero biases"

    np_mm = np.float32 if mm_mode == "f32r" else ml_dtypes.bfloat16

    g0 = _g_host(h_0[:, 0, :])  # [B, H]
    xTs = [np.ascontiguousarray(x[b].T).astype(np_mm) for b in range(B)]

    in_maps = []
    for c in range(NCORES):
        b, hh = divmod(c, 2)
        hs = slice(hh * HS, (hh + 1) * HS)
        wcat = np.concatenate([W_f[:, hs], W_i[:, hs], W_h[:, hs]],
                              axis=1).astype(np_mm)
        auxa = np.ascontiguousarray(
            g0[b, hs].reshape(NHT, 128).T.astype(np.float32))
        in_maps.append({"xT": xTs[b], "w": wcat, "aux": auxa})

    nc = _get_nc(mm_mode)
    res = run_bass_kernel_spmd(nc, in_maps, core_ids=list(range(NCORES)),
                               trace=trace)

    out = np.empty((B, T + 1, H), np.float32)
    out[:, 0, :] = g0
    for c in range(NCORES):
        b, hh = divmod(c, 2)
        hs = slice(hh * HS, (hh + 1) * HS)
        out[b, 1:, hs] = res.results[c]["out"].T
    return out, res


def kernel(**inputs):
    out, _ = _run(inputs)
    return out



# revision 9
# speedup vs baseline: 1.0977x; 1.0977x over previous
"""MinLSTM Trainium2 kernel — fp8 DoubleRow matmuls + fused DVE gates.

Full-input contract: kernel(**inputs) takes the complete (unsharded) numpy
inputs of the reference model and returns the full [B, T+1, H] float32 output.

Math (per batch b, channel h — identical to the reference's log-space scan,
computed in linear space; every quantity is positive so the linear recurrence
is numerically stable):
    a = x @ W_f ;  b = x @ W_i ;  c = x @ W_h          (zero biases)
    f = sigmoid(a) / (sigmoid(a) + sigmoid(b))          # forget gate
    g = max(c + 0.5, sigmoid(c))                        # = exp(log_g(c))
    h_t = f_t h_{t-1} + (1 - f_t) g_t,  h_{-1} = g(h_0)

Sharding: 8 cores, core c -> (sample b = c//2, H-half hh = c%2, 256 channels).
Fully independent cores, no collectives.

Device pipeline per core (T chunks of 512, pairs of chunks batched for the
elementwise stages):
  PE    fp8-e4m3 DoubleRow matmuls (0.5 cyc/row): a,b from x8; c from
        x8*W8h + dx8*W8h + x8*dW8h (x- and W-residual streams make the
        c-projection ~bf16-accurate; a,b tolerate plain fp8). The +0.5*beta
        bias of c rides a sacrificial constant-1 channel of the dx8 stream.
  ACT   one sigmoid pass over the [a;b] PSUM pair -> sa, sb (f32 SBUF)
  DVE   two fused custom ops:
          F:  f = sa * recip1(sa+sb)        (bitwise-NOT seed + linear poly)
          V:  vbar = (f-1) * max(m, q2(min(m,1))^2),  m = beta*(c+0.5) PSUM
  Pool  tensor_tensor_scan: state = f*state - vbar  (f32 state, f16 io)
  DMA   h~ = beta*h written as f16; host divides by beta and transposes.

beta (=1.75) rescales the c-gate so the sigma-branch quadratic fits in the
custom op's 3 scalar slots; the scan is linear in (v, init) so scaling g0 by
beta scales h exactly.
"""

from contextlib import ExitStack

import numpy as np
import ml_dtypes

import concourse.bacc as bacc
import concourse.tile as tile
import concourse.mybir as mybir
from concourse.bass_utils import run_bass_kernel_spmd

import concourse.dve_ops as _dve_ops
from concourse.dve_spec import (Spec as _Spec, Src0 as _S0, Src1 as _S1,
                                C0 as _C0, C1 as _C1, C2 as _C2, One as _One,
                                AluOp as _AluOp, Bin as _Bin, maxx as _maxx,
                                minn as _minn, lower as _lower)
from concourse.dve_uop import DveOpSpec as _DveOpSpec
from concourse.dve_table_gen import dve_ver_for as _dve_ver_for

F8 = mybir.dt.float8e4
F16 = mybir.dt.float16
F32 = mybir.dt.float32
AF = mybir.ActivationFunctionType
OP = mybir.AluOpType
DR = mybir.MatmulPerfMode.DoubleRow
NPF8 = ml_dtypes.float8_e4m3

B, T, D, H = 4, 8192, 512, 512
NCORES = 8
HS = H // 2          # 256 channels per core
TC = 512             # matmul T-chunk width
NCH = T // TC        # 16 chunks
NSUP = NCH // 2      # 8 superblocks (2 chunks each) for the DVE/scan stages
TW = 2 * TC          # superblock width
NHT = HS // 128      # 2 h-tiles per core

BETA = 1.75
# f = sa * r, r ~ 1/(sa+sb): NOT-seed nx, u = x*nx in [-4.5,-4], r=(u*c0+c1)*nx
F_CONSTS = (-0.05560890019581849, -0.4720664899356389)
# vbar = (f-1)*max(m, q(min(m,1))^2), q = (m*k0+k1)*m+k2 fits
# sqrt(beta*sigmoid(m/beta-0.5)) on m in [-6*beta, 1]
V_CONSTS = (0.007289407906601352, 0.14709027872923935, 0.802180149132902)


def _register_op(name, body, ref, rd1=True):
    if name in _dve_ops._SUB_OPCODE_FOR_NAME:
        return next(o for o in _dve_ops.OPS if o.name == name)
    spec = _Spec(body=body, reference=ref)
    row = _dve_ops._CUSTOM_DVE_ROW_BASE + len(_dve_ops.OPS)
    assert row < 0x20
    ver = _dve_ver_for("TRN2")
    sha = _DveOpSpec(name=name, opcode=row, uops=_lower(spec, ver=ver),
                     rd1_en=rd1).sha(ver)
    op = _dve_ops.DveOp(name, spec, subdim=False, uops_sha={ver: sha})
    _dve_ops.OPS.append(op)
    _dve_ops.CUSTOM_DVE_SPECS[name] = spec
    _dve_ops._SUB_OPCODE_FOR_NAME[name] = row
    return op


def _f_ref(in0, in1, c0, c1, c2):
    sa = np.asarray(in0, np.float32)
    x = (sa + np.asarray(in1, np.float32)).astype(np.float32)
    nx = (~x.view(np.int32)).view(np.float32)
    u = x * nx
    return sa * ((u * c0 + c1) * nx)


def _g_ref(in0, in1, c0, c1, c2):
    m = np.asarray(in0, np.float32)
    mc = np.minimum(m, np.float32(1.0))
    q = (mc * c0 + c1) * mc + c2
    return np.maximum(m, q * q).astype(np.float32)


def _build_f_op():
    x = _S0 + _S1
    nx = _Bin(_AluOp.BITWISE_NOT, x, x)
    u = x * nx
    body = _S0 * ((u * _C0 + _C1) * nx)
    return _register_op("MINLSTM_FGATE_ANT", body, _f_ref)


def _build_g_op():
    m = _minn(_S0, _One)
    q = (m * _C0 + _C1) * m + _C2
    body = _maxx(_S0, q * q)
    return _register_op("MINLSTM_GGATE_ANT", body, _g_ref, rd1=False)


_F_OP = _build_f_op()
_G_OP = _build_g_op()

_nc_cache = {}


def _build_nc():
    nc = bacc.Bacc("TRN2", target_bir_lowering=False, debug=False,
                   num_devices=NCORES)
    # xin rows: idx = s*4 + kg*2 + i  (s: 0=x8 1=dx8; kg: k-group; i: pair)
    xin = nc.dram_tensor("xin", [128, 8, T], F8, kind="ExternalInput")
    # wab rows: idx = ((kg*2 + g)*2 + ht)*2 + i, cols m   (g: 0=W_f 1=W_i)
    wab = nc.dram_tensor("wab", [128, 16, 128], F8, kind="ExternalInput")
    # wc rows: idx = ((kg*3 + role)*2 + ht)*2 + i  (role: 0=W8h 1=Wdx 2=dWh8)
    wc = nc.dram_tensor("wc", [128, 24, 128], F8, kind="ExternalInput")
    aux = nc.dram_tensor("aux", [128, NHT], F32, kind="ExternalInput")
    out = nc.dram_tensor("out", [128, NHT, T], F16, kind="ExternalOutput")

    with tile.TileContext(nc) as tc, ExitStack() as ctx:
        wpool = ctx.enter_context(tc.tile_pool(name="w", bufs=1))
        xpool = ctx.enter_context(tc.tile_pool(name="x", bufs=3))
        spool = ctx.enter_context(tc.tile_pool(name="s", bufs=3))
        gpool = ctx.enter_context(tc.tile_pool(name="g", bufs=3))
        hpool = ctx.enter_context(tc.tile_pool(name="h", bufs=3))
        ppool = ctx.enter_context(tc.tile_pool(name="p", bufs=2, space="PSUM"))

        wab_t = wpool.tile([128, 16, 128], F8, tag="wab")
        nc.scalar.dma_start(wab_t[:], wab[:])
        wc_t = wpool.tile([128, 24, 128], F8, tag="wc")
        nc.scalar.dma_start(wc_t[:], wc[:])
        auxt = wpool.tile([128, NHT], F32, tag="aux")
        nc.scalar.dma_start(auxt[:], aux[:])

        carry = [None] * NHT
        for sup in range(NSUP):
            pcs, sabs = [], []
            for ht in range(NHT):
                pcs.append(ppool.tile([128, 2, TC], F32, tag="pc", bufs=2,
                                      name=f"pc{ht}"))
                sabs.append(spool.tile([128, 2, TW], F32, tag="sab", bufs=3,
                                       name=f"sab{ht}"))
            for e in range(2):
                ci = 2 * sup + e
                tsl = slice(ci * TC, (ci + 1) * TC)
                xt = xpool.tile([128, 8, TC], F8, tag="x", name="x")
                nc.sync.dma_start(xt[:], xin[:, :, tsl])

                def rhs(s, kg):
                    r0 = s * 4 + kg * 2
                    return xt[:, r0:r0 + 2, :]

                for ht in range(NHT):
                    pfi = ppool.tile([128, 2, TC], F32, tag="pfi", bufs=2)
                    for g in range(2):
                        for kg in range(2):
                            w0 = ((kg * 2 + g) * 2 + ht) * 2
                            nc.tensor.matmul(
                                pfi[:, g, :], wab_t[:, w0:w0 + 2, :],
                                rhs(0, kg), start=(kg == 0), stop=(kg == 1),
                                perf_mode=DR)
                    cparts = [(0, 0), (1, 1), (2, 0)]  # (role, stream)
                    for pi, (role, s) in enumerate(cparts):
                        for kg in range(2):
                            w0 = ((kg * 3 + role) * 2 + ht) * 2
                            nc.tensor.matmul(
                                pcs[ht][:, e, :], wc_t[:, w0:w0 + 2, :],
                                rhs(s, kg),
                                start=(pi == 0 and kg == 0),
                                stop=(pi == 2 and kg == 1), perf_mode=DR)
                    nc.scalar.activation(sabs[ht][:, :, e * TC:(e + 1) * TC],
                                         pfi[:, :, :], AF.Sigmoid)
            tsl2 = slice(sup * TW, (sup + 1) * TW)
            for ht in range(NHT):
                f_t = gpool.tile([128, TW], F16, tag=f"f{ht}", name="f")
                nc.vector._custom_dve(_F_OP, out=f_t[:],
                                      in0=sabs[ht][:, 0, :],
                                      in1=sabs[ht][:, 1, :],
                                      s0=F_CONSTS[0], s1=F_CONSTS[1])
                g_t = gpool.tile([128, TW], F16, tag=f"g{ht}", name="g")
                nc.vector._custom_dve(_G_OP, out=g_t[:],
                                      in0=pcs[ht][:, :, :],
                                      s0=V_CONSTS[0], s1=V_CONSTS[1],
                                      imm2=V_CONSTS[2])
                vb_t = gpool.tile([128, TW], F16, tag=f"v{ht}", name="v")
                nc.vector.scalar_tensor_tensor(vb_t[:], f_t[:], 1.0, g_t[:],
                                               OP.subtract, OP.mult)
                h_t = hpool.tile([128, TW], F16, tag=f"h{ht}", name="h")
                ini = auxt[:, ht:ht + 1] if sup == 0 else carry[ht]
                nc.vector.tensor_tensor_scan(h_t[:], f_t[:], vb_t[:], ini,
                                             OP.mult, OP.subtract)
                carry[ht] = h_t[:, TW - 1:TW]
                nc.sync.dma_start(out[:, ht, tsl2], h_t[:])
    nc.compile()
    return nc


def _get_nc():
    if "nc" not in _nc_cache:
        _nc_cache["nc"] = _build_nc()
    return _nc_cache["nc"]


def _g_host(x):
    # exp(log_g(x)) of the reference, computed directly in fp32
    return np.where(x >= 0, x + 0.5, 1.0 / (1.0 + np.exp(-np.minimum(x, 0))))


def _pack_dpairs(mat):
    """[T-or-D rows laid out d, cols] -> rows reindexed (kg, k, i): d = kg*256 + 2k + i.
    Input [D, N] -> output [128, 2kg, 2i, N] with out[k, kg, i] = in[kg*256+2k+i]."""
    m = mat.reshape(2, 128, 2, -1)          # [kg, k, i, N]
    return np.ascontiguousarray(m.transpose(1, 0, 2, 3))  # [k, kg, i, N]


def _run(inputs, trace=False):
    x = np.asarray(inputs["x"], np.float32)
    h_0 = np.asarray(inputs["h_0"], np.float32)
    W = {k: np.asarray(inputs[k], np.float32)
         for k in ("W_f", "W_i", "W_h")}
    for k in ("b_f", "b_i", "b_h"):
        assert (np.asarray(inputs[k]) == 0).all(), \
            "device program folds zero biases"

    g0 = _g_host(h_0[:, 0, :])  # [B, H]

    # --- x streams, packed once per sample ------------------------------
    xins = []
    for b in range(B):
        x8 = x[b].astype(NPF8)                       # [T, D]
        dx = (x[b] - x8.astype(np.float32)).astype(NPF8)
        # [2s, T, D] -> rows (k, s, kg, i) cols T
        s_td = np.stack([x8, dx], axis=0)            # [2, T, D]
        m = s_td.reshape(2, T, 2, 128, 2)            # [s, T, kg, k, i]
        m = m.transpose(3, 0, 2, 4, 1)               # [k, s, kg, i, T]
        m = np.ascontiguousarray(m).reshape(128, 8, T)
        m[127, 7, :] = NPF8(1.0)                     # bias slot: s=1,kg=1,i=1
        xins.append(m)

    in_maps = []
    for c in range(NCORES):
        b, hh = divmod(c, 2)
        hs = slice(hh * HS, (hh + 1) * HS)
        w8f = _pack_dpairs(W["W_f"][:, hs].astype(NPF8))   # [128,2,2,256]
        w8i = _pack_dpairs(W["W_i"][:, hs].astype(NPF8))
        whb = BETA * W["W_h"][:, hs]
        w8h = whb.astype(NPF8)
        dwh = (whb - w8h.astype(np.float32)).astype(NPF8)
        w8h_p = _pack_dpairs(w8h)
        dwh_p = _pack_dpairs(dwh)
        wdx_p = w8h_p.copy()
        wdx_p[127, 1, 1, :] = NPF8(0.5 * BETA)       # bias row (d=511)
        # wab rows: ((kg*2+g)*2+ht)*2+i ; cols m in [0,128)
        wab = np.zeros((128, 16, 128), NPF8)
        wcm = np.zeros((128, 24, 128), NPF8)
        for kg in range(2):
            for i in range(2):
                for ht in range(NHT):
                    mcols = slice(ht * 128, (ht + 1) * 128)
                    for g, wp in ((0, w8f), (1, w8i)):
                        wab[:, ((kg * 2 + g) * 2 + ht) * 2 + i, :] = \
                            wp[:, kg, i, mcols]
                    for role, wp in ((0, w8h_p), (1, wdx_p), (2, dwh_p)):
                        wcm[:, ((kg * 3 + role) * 2 + ht) * 2 + i, :] = \
                            wp[:, kg, i, mcols]
        auxa = np.ascontiguousarray(
            (BETA * g0[b, hs]).reshape(NHT, 128).T.astype(np.float32))
        in_maps.append({"xin": xins[b], "wab": wab, "wc": wcm, "aux": auxa})

    nc = _get_nc()
    res = run_bass_kernel_spmd(nc, in_maps, core_ids=list(range(NCORES)),
                               trace=trace)

    out = np.empty((B, T + 1, H), np.float32)
    out[:, 0, :] = g0
    inv_beta = np.float32(1.0 / BETA)
    for c in range(NCORES):
        b, hh = divmod(c, 2)
        o = np.asarray(res.results[c]["out"], np.float32)  # [128, NHT, T]
        # channel = hh*256 + ht*128 + p
        out[b, 1:, hh * HS:(hh + 1) * HS] = \
            (o.transpose(2, 1, 0).reshape(T, HS)) * inv_beta
    return out, res


def kernel(**inputs):
    out, _ = _run(inputs)
    return out
